# revision 1
# baseline (speedup 1.0000x reference)
"""HGCN layer kernel for Trainium2, 8 NeuronCores, row-sharded SPMD.

Reference computation (N=6144, D=512):
    type_sum_a = adj_a @ x ; type_sum_b = adj_b @ x
    attn_a = sigmoid(cat[ts_a, x] @ Wa.T + ba) ; attn_b likewise
    h = x @ W_sa ; s_l = h @ a_sa[:512] ; s_r = h @ a_sa[512:]
    scores[i,j] = s_l[i] + s_r[j]
    e = adj_a * exp(-leaky_relu(scores, 0.01)) ; attn = e / (rowsum(e)+1e-5)
    x_a = attn @ h ; x_b = adj_b @ (x @ W_gcnb) + b_gcnb
    out = sigmoid(attn_a * x_a + attn_b * x_b)

Kernel strategy (per core, NL=768 local rows):
  - R = [W_sa | W_gcnb | W_sa@a_l | W_sa@a_r | Wa1.T | Wb1.T | Wa2.T | Wb2.T]
    HX = x @ R computed replicated on every core (needs full h/xW anyway).
    Gates reassociate (adj@x)@W1.T -> adj@(x@W1.T) so the NxN gate matmuls
    shrink to N-vector contractions folded into PE side-passes.
  - e computed in transposed layout [j(part), i(free)] so it is directly the
    lhsT of the attention matmul; adjacency is passed in pre-transposed,
    per-core-permuted (local rows first) so one SPMD program serves all cores.
  - rowsum(e) via ones-vector lhsT pass; division applied after the matmul.
  - float32r matmuls (4x fp32 PE rate at N>=256).
"""

import numpy as np
from contextlib import ExitStack

import concourse.bass as bass
import concourse.bacc as bacc
import concourse.mybir as mybir
import concourse.tile as tile

F32 = mybir.dt.float32
F32R = mybir.dt.float32r
BF16 = mybir.dt.bfloat16
AF = mybir.ActivationFunctionType
ALU = mybir.AluOpType

N_CORES = 8


def _chunks(total, size=512):
    out = []
    o = 0
    while o < total:
        out.append((o, min(size, total - o)))
        o += size
    return out


def build_program(n, d, nl, ba, bb, dt_a=F32R, dt_bc=BF16,
                  lrelu_on_act=False):
    """Build the SPMD Bass program. Returns nc.

    n: total nodes, d: feature dim, nl: local rows per core.
    ba/bb: python-float gate biases (baked in).
    """
    JT = n // 128   # j tiles (contraction/node axis)
    IT = nl // 128  # local row tiles
    KT = d // 128   # feature k tiles
    NR = 2 * d + 8  # columns of R
    # stats cols: 0=s_l 1=s_r 2=zero 3=va 4=vb 5=wa2x 6=wb2x 7=pad

    nc = bacc.Bacc("TRN2", target_bir_lowering=False, debug=False,
                   num_devices=N_CORES)

    xt_dram = nc.dram_tensor("xt", [JT, KT, 128, 128], dt_a, kind="ExternalInput")
    r_dram = nc.dram_tensor("rmat", [KT, 128, NR], dt_a, kind="ExternalInput")
    adjat_dram = nc.dram_tensor("adjat", [JT, 128, nl], dt_bc, kind="ExternalInput")
    adjbt_dram = nc.dram_tensor("adjbt", [JT, 128, nl], dt_bc, kind="ExternalInput")
    bbias_dram = nc.dram_tensor("bbias", [128, d], F32, kind="ExternalInput")
    ident_dram = nc.dram_tensor("ident", [128, 128], F32, kind="ExternalInput")
    out_dram = nc.dram_tensor("out", [nl, d], F32, kind="ExternalOutput")

    xw_resident = mybir.dt.size(dt_bc) <= 2
    xw_dram = None
    if not xw_resident:
        xw_dram = nc.dram_tensor("xw_scratch", [JT, 128, d], dt_bc)

    def mm(out, lhsT, rhs, start, stop, skip_group_check=False):
        nc.tensor.matmul(out, lhsT, rhs, start=start, stop=stop,
                         skip_group_check=skip_group_check)

    with tile.TileContext(nc) as tc, ExitStack() as ctx:
        const = ctx.enter_context(tc.tile_pool(name="const", bufs=1))

        r_sb = const.tile([128, KT, NR], dt_a, tag="r")
        h_sb = const.tile([128, JT * d], dt_bc, tag="h")
        xw_sb = None
        if xw_resident:
            xw_sb = const.tile([128, JT * d], dt_bc, tag="xw", name="xw_sb")
        stats_sb = const.tile([128, JT * 8], F32, tag="stats")
        stats_r = const.tile([128, JT * 8], dt_bc, tag="stats_r")
        slb_sb = const.tile([128, nl], F32, tag="slb")
        xb_sb = const.tile([128, IT * d], F32, tag="xb")
        xa_sb = const.tile([128, IT * d], F32, tag="xa")
        bbias_sb = const.tile([128, d], F32, tag="bbias")
        ident_sb = const.tile([128, 128], F32, tag="ident")
        onespad = const.tile([128, 2], dt_bc, tag="onespad")
        onespad_f = const.tile([128, 2], F32, tag="onespadf")
        ones_row = const.tile([1, 128], F32, tag="ones_r")
        neg1 = const.tile([128, 1], F32, tag="neg1")
        ba_sb = const.tile([128, 1], F32, tag="ba")
        bb_sb = const.tile([128, 1], F32, tag="bb")
        sl_row = const.tile([1, nl], F32, tag="sl_row")
        g_sb = const.tile([128, 3 * IT], F32, tag="g")  # rs|ga|gb cols
        rg_rows = const.tile([2, nl], F32, tag="rg_rows")  # row0=rs row1=ga
        gb_row = const.tile([1, nl], F32, tag="gb_row")
        gate_sb = const.tile([128, 4 * IT], F32, tag="gate")
        # gate_sb cols: [0:IT]=recip(rowsum), [IT:2IT]=sig_a, [2IT:3IT]=sig_b,
        # [3IT:4IT]=scratch

        for k in range(KT):
            nc.sync.dma_start(out=r_sb[:, k, :], in_=r_dram[k])
        nc.sync.dma_start(out=bbias_sb[:], in_=bbias_dram[:])
        nc.sync.dma_start(out=ident_sb[:], in_=ident_dram[:])
        nc.vector.memset(onespad_f[:], 0.0)
        nc.vector.memset(onespad_f[:, 0:1], 1.0)
        nc.vector.tensor_copy(onespad[:], onespad_f[:])
        nc.vector.memset(ones_row[:], 1.0)
        nc.vector.memset(neg1[:], -1.0)
        nc.vector.memset(ba_sb[:], float(ba))
        nc.vector.memset(bb_sb[:], float(bb))

        # ---- Phase A: HX = x @ R (replicated over all n nodes) ----
        with tc.tile_pool(name="xt_pool", bufs=3) as xtp, \
             tc.tile_pool(name="xw_out", bufs=3) as xwop, \
             tc.tile_pool(name="psA", bufs=2, space="PSUM") as psA:
            for m in range(JT):
                xt_t = xtp.tile([128, KT * 128], dt_a, tag="xt")
                for k in range(KT):
                    nc.sync.dma_start(out=xt_t[:, k * 128:(k + 1) * 128],
                                      in_=xt_dram[m, k])
                ph = psA.tile([128, d], F32, tag="ph")
                pw = psA.tile([128, d], F32, tag="pw")
                ps = psA.tile([128, 8], F32, tag="ps")
                for k in range(KT):
                    lhsT = xt_t[:, k * 128:(k + 1) * 128]
                    st, sp = (k == 0), (k == KT - 1)
                    mm(ph[:], lhsT, r_sb[:, k, 0:d], st, sp)
                    mm(pw[:], lhsT, r_sb[:, k, d:2 * d], st, sp)
                    mm(ps[:], lhsT, r_sb[:, k, 2 * d:NR], st, sp)
                nc.scalar.copy(h_sb[:, m * d:(m + 1) * d], ph[:])
                if xw_resident:
                    nc.scalar.copy(xw_sb[:, m * d:(m + 1) * d], pw[:])
                else:
                    xw_t = xwop.tile([128, d], dt_bc, tag="xwo")
                    nc.scalar.copy(xw_t[:], pw[:])
                    nc.sync.dma_start(out=xw_dram[m], in_=xw_t[:])
                nc.vector.tensor_copy(stats_sb[:, m * 8:(m + 1) * 8], ps[:])
                nc.vector.tensor_copy(stats_r[:, m * 8:(m + 1) * 8], ps[:])

        # ---- Phase A2: build SL broadcast [128, nl] from local s_l ----
        with tc.tile_pool(name="psA2", bufs=1, space="PSUM") as psA2:
            ch = _chunks(nl)
            ptrs = [psA2.tile([1, c[1]], F32, tag=f"psl{ci}",
                              name=f"psl{ci}")
                    for ci, c in enumerate(ch)]
            for t in range(IT):
                ci, off = divmod(t * 128, 512)
                # transpose stats col (s_l of local tile t) -> row chunk
                nc.tensor.matmul(ptrs[ci][0:1, off:off + 128],
                                 stats_sb[:, t * 8:t * 8 + 1],
                                 ident_sb[:], start=True, stop=True)
            for ci, (o, w) in enumerate(ch):
                nc.vector.tensor_copy(sl_row[0:1, o:o + w], ptrs[ci][0:1, :])
            for ci, (o, w) in enumerate(ch):
                pb = psA2.tile([128, w], F32, tag="pslb")
                nc.tensor.matmul(pb[:], ones_row[:], sl_row[0:1, o:o + w],
                                 start=True, stop=True)
                nc.vector.tensor_copy(slb_sb[:, o:o + w], pb[:])

        # ---- Phase B: x_b = adj_b @ xW ; gb = vb^T adj_bT ----
        # gb done row-oriented: lhsT is the tiny vb vector (cheap weight
        # load), adjacency streams as the moving operand.
        with tc.tile_pool(name="adjB", bufs=5) as adjp, \
             tc.tile_pool(name="rhsB", bufs=3) as rhsp, \
             tc.tile_pool(name="psB", bufs=1, space="PSUM") as psB:
            pb_acc = [psB.tile([128, d], F32, tag=f"pb{i}", name=f"pb{i}")
                      for i in range(IT)]
            chn = _chunks(nl)
            pgb = [psB.tile([1, c[1]], F32, tag=f"pgb{ci}", name=f"pgb{ci}")
                   for ci, c in enumerate(chn)]
            for j in range(JT):
                at = adjp.tile([128, nl], dt_bc, tag="adj")
                nc.sync.dma_start(out=at[:], in_=adjbt_dram[j])
                if xw_resident:
                    xw_t = xw_sb[:, j * d:(j + 1) * d]
                else:
                    xw_t = rhsp.tile([128, d], dt_bc, tag="xw")
                    nc.sync.dma_start(out=xw_t[:], in_=xw_dram[j])
                st, sp = (j == 0), (j == JT - 1)
                vb = stats_r[:, j * 8 + 4:j * 8 + 5]
                for i in range(IT):
                    mm(pb_acc[i][:], at[:, i * 128:(i + 1) * 128], xw_t[:], st, sp)
                for ci, (o, w) in enumerate(chn):
                    mm(pgb[ci][:], vb, at[:, o:o + w], st, sp)
            for i in range(IT):
                nc.scalar.copy(xb_sb[:, i * d:(i + 1) * d], pb_acc[i][:])
            for ci, (o, w) in enumerate(chn):
                nc.vector.tensor_copy(gb_row[0:1, o:o + w], pgb[ci][0:1, :])

        # ---- Phase C: e = adj_a * exp(-lrelu(s)); y_a = e^T.T @ h ----
        # rowsum and ga are row-oriented with zero-padded M=2 weights:
        # pass1 lhsT=[1|0] rhs=e -> row0 += rowsum; pass2 lhsT=[0|va]
        # rhs=adj -> row1 += ga. Disjoint rows of one accumulator pair.
        with tc.tile_pool(name="adjC", bufs=5) as adjp, \
             tc.tile_pool(name="ewC", bufs=4) as ewp, \
             tc.tile_pool(name="psC", bufs=1, space="PSUM") as psC:
            pc_acc = [psC.tile([128, d], F32, tag=f"pc{i}", name=f"pc{i}")
                      for i in range(IT)]
            chn = _chunks(nl)
            prg = [psC.tile([2, c[1]], F32, tag=f"prg{ci}", name=f"prg{ci}")
                   for ci, c in enumerate(chn)]
            for j in range(JT):
                at = adjp.tile([128, nl], dt_bc, tag="adj")
                nc.sync.dma_start(out=at[:], in_=adjat_dram[j])
                s_r = stats_sb[:, j * 8 + 1:j * 8 + 2]
                m_t = ewp.tile([128, nl], F32, tag="m")
                if lrelu_on_act:
                    nc.scalar.activation(m_t[:], slb_sb[:], AF.Prelu,
                                         bias=s_r, alpha=0.01)
                else:
                    nc.vector.tensor_scalar_add(m_t[:], slb_sb[:], s_r)
                    nc.vector.scalar_tensor_tensor(m_t[:], m_t[:], 0.01, m_t[:],
                                                   op0=ALU.mult, op1=ALU.max)
                # w = exp(-m), in place
                nc.scalar.activation(m_t[:], m_t[:], AF.Exp, scale=neg1[:])
                e_t = ewp.tile([128, nl], dt_bc, tag="e")
                nc.vector.tensor_tensor(e_t[:], m_t[:], at[:], op=ALU.mult)
                st, sp = (j == 0), (j == JT - 1)
                zva = stats_r[:, j * 8 + 2:j * 8 + 4]
                for i in range(IT):
                    mm(pc_acc[i][:], e_t[:, i * 128:(i + 1) * 128],
                       h_sb[:, j * d:(j + 1) * d], st, sp)
                for ci, (o, w) in enumerate(chn):
                    mm(prg[ci][:], onespad[:], e_t[:, o:o + w], st, False)
                    mm(prg[ci][:], zva, at[:, o:o + w], False, sp)
            for i in range(IT):
                nc.scalar.copy(xa_sb[:, i * d:(i + 1) * d], pc_acc[i][:])
            for ci, (o, w) in enumerate(chn):
                nc.vector.tensor_copy(rg_rows[0:2, o:o + w], prg[ci][0:2, :])

        # ---- Phase D: transpose stat rows to columns, gates, combine ----
        with tc.tile_pool(name="psD", bufs=1, space="PSUM") as psD, \
             tc.tile_pool(name="outD", bufs=2) as outp:
            pT = psD.tile([128, 3 * IT], F32, tag="pT")
            for i in range(IT):
                # transpose [rs; ga] pair: K=2 against 2x2 identity
                nc.tensor.matmul(pT[:, 2 * i:2 * i + 2],
                                 rg_rows[0:2, i * 128:(i + 1) * 128],
                                 ident_sb[0:2, 0:2], start=True, stop=True)
                nc.tensor.matmul(pT[:, 2 * IT + i:2 * IT + i + 1],
                                 gb_row[0:1, i * 128:(i + 1) * 128],
                                 ones_row[0:1, 0:1], start=True, stop=True)
            nc.vector.tensor_copy(g_sb[:], pT[:])
            for i in range(IT):
                # recip(rowsum + 1e-5)
                nc.vector.tensor_scalar_add(gate_sb[:, 3 * IT + i:3 * IT + i + 1],
                                            g_sb[:, 2 * i:2 * i + 1], 1e-5)
                nc.vector.reciprocal(gate_sb[:, i:i + 1],
                                     gate_sb[:, 3 * IT + i:3 * IT + i + 1])
                # sig_a = sigmoid(ga + wa2x + ba)
                nc.vector.tensor_tensor(gate_sb[:, 3 * IT + i:3 * IT + i + 1],
                                        g_sb[:, 2 * i + 1:2 * i + 2],
                                        stats_sb[:, i * 8 + 5:i * 8 + 6],
                                        op=ALU.add)
                nc.scalar.activation(gate_sb[:, IT + i:IT + i + 1],
                                     gate_sb[:, 3 * IT + i:3 * IT + i + 1],
                                     AF.Sigmoid, bias=ba_sb[:])
                # sig_b = sigmoid(gb + wb2x + bb)
                nc.vector.tensor_tensor(gate_sb[:, 3 * IT + i:3 * IT + i + 1],
                                        g_sb[:, 2 * IT + i:2 * IT + i + 1],
                                        stats_sb[:, i * 8 + 6:i * 8 + 7],
                                        op=ALU.add)
                nc.scalar.activation(gate_sb[:, 2 * IT + i:2 * IT + i + 1],
                                     gate_sb[:, 3 * IT + i:3 * IT + i + 1],
                                     AF.Sigmoid, bias=bb_sb[:])
            for i in range(IT):
                u_t = outp.tile([128, d], F32, tag="u")
                # u = sig_a * (x_a_raw * recip)
                nc.vector.tensor_scalar(u_t[:], xa_sb[:, i * d:(i + 1) * d],
                                        gate_sb[:, i:i + 1],
                                        gate_sb[:, IT + i:IT + i + 1],
                                        op0=ALU.mult, op1=ALU.mult)
                t_t = outp.tile([128, d], F32, tag="t")
                # t = x_b_raw + b_gcnb
                nc.vector.tensor_tensor(t_t[:], xb_sb[:, i * d:(i + 1) * d],
                                        bbias_sb[:], op=ALU.add)
                # y = sigmoid(t * sig_b + u)
                nc.vector.scalar_tensor_tensor(t_t[:], t_t[:],
                                               gate_sb[:, 2 * IT + i:2 * IT + i + 1],
                                               u_t[:], op0=ALU.mult, op1=ALU.add)
                y_t = outp.tile([128, d], F32, tag="y")
                nc.scalar.activation(y_t[:], t_t[:], AF.Sigmoid)
                nc.sync.dma_start(out=out_dram[i * 128:(i + 1) * 128, :],
                                  in_=y_t[:])

    nc.compile()
    return nc


def make_r_matrix(W_sa, a_sa, W_gcnb, Wa, Wb, d):
    cols = np.zeros((d, 8), dtype=np.float32)
    cols[:, 0] = W_sa @ a_sa[0, :d]
    cols[:, 1] = W_sa @ a_sa[0, d:]
    # col 2 stays zero (zero-pad for the [0|va] gate weight pair)
    cols[:, 3] = Wa[0, :d]
    cols[:, 4] = Wb[0, :d]
    cols[:, 5] = Wa[0, d:]
    cols[:, 6] = Wb[0, d:]
    return np.ascontiguousarray(
        np.concatenate([W_sa, W_gcnb, cols], axis=1)).astype(np.float32)


def make_core_inputs(x, adj_a, adj_b, R, b_gcnb, n, d, nl, core,
                     np_a=np.float32, np_bc=None):
    if np_bc is None:
        import ml_dtypes
        np_bc = ml_dtypes.bfloat16
    JT, KT = n // 128, d // 128
    rows = np.arange(core * nl, (core + 1) * nl)
    perm = np.concatenate([rows, np.arange(0, core * nl),
                           np.arange((core + 1) * nl, n)])
    xp = x[perm]
    xt = np.ascontiguousarray(
        xp.reshape(JT, 128, KT, 128).transpose(0, 2, 3, 1))
    adjat = np.ascontiguousarray(adj_a[rows][:, perm].T).reshape(JT, 128, nl)
    adjbt = np.ascontiguousarray(adj_b[rows][:, perm].T).reshape(JT, 128, nl)
    return {
        "xt": xt.astype(np_a),
        "rmat": R.reshape(KT, 128, 2 * d + 8).astype(np_a),
        "adjat": adjat.astype(np_bc),
        "adjbt": adjbt.astype(np_bc),
        "bbias": np.ascontiguousarray(
            np.broadcast_to(b_gcnb, (128, d))).astype(np.float32),
        "ident": np.eye(128, dtype=np.float32),
    }


_CACHE = {}


def _install_ntff_hook():
    """Dev-only: register the axon NTFF profile hook so trace=True works.

    The agent image's antenv package lacks axon_hooks; synthesize it and
    wire trn_boot's ctypes-based hook to /opt/axon/libaxon_pjrt.so.
    """
    import sys
    import types
    try:
        from antenv import axon_hooks  # noqa: F401
        return
    except ImportError:
        pass
    import antenv
    mod = types.ModuleType("antenv.axon_hooks")
    _h = [None]
    mod.get_axon_ntff_profile_hook = lambda: _h[0]
    mod.set_axon_ntff_profile_hook = lambda hook: _h.__setitem__(0, hook)
    sys.modules["antenv.axon_hooks"] = mod
    antenv.axon_hooks = mod
    from trn_agent_boot.trn_boot import _ntff_profile_via_ctypes
    mod.set_axon_ntff_profile_hook(
        _ntff_profile_via_ctypes("/opt/axon/libaxon_pjrt.so"))


def kernel(x, adj_a, adj_b, W_sa, a_sa, W_gcnb, b_gcnb, Wa, ba, Wb, bb,
           _trace=False, _trace_kwargs=None):
    from concourse.bass_utils import run_bass_kernel_spmd
    if _trace:
        _install_ntff_hook()

    n, d = x.shape
    nl = n // N_CORES
    R = make_r_matrix(W_sa, a_sa, W_gcnb, Wa, Wb, d)

    key = (n, d, nl, float(ba[0]), float(bb[0]))
    if key not in _CACHE:
        _CACHE[key] = build_program(n, d, nl, float(ba[0]), float(bb[0]))
    nc = _CACHE[key]

    in_maps = [make_core_inputs(x, adj_a, adj_b, R, b_gcnb, n, d, nl, c)
               for c in range(N_CORES)]
    res = run_bass_kernel_spmd(nc, in_maps, list(range(N_CORES)),
                               trace=_trace, **(_trace_kwargs or {}))
    out = np.empty((n, d), dtype=np.float32)
    for c in range(N_CORES):
        out[c * nl:(c + 1) * nl] = res.results[c]["out"]
    if _trace:
        kernel._last_results = res
    return out



# revision 30
# speedup vs baseline: 1.3994x; 1.3994x over previous
"""HGCN layer kernel for Trainium2, 8 NeuronCores, row-sharded SPMD.

Reference computation (N=6144, D=512):
    type_sum_a = adj_a @ x ; type_sum_b = adj_b @ x
    attn_a = sigmoid(cat[ts_a, x] @ Wa.T + ba) ; attn_b likewise
    h = x @ W_sa ; s_l = h @ a_sa[:512] ; s_r = h @ a_sa[512:]
    scores[i,j] = s_l[i] + s_r[j]
    e = adj_a * exp(-leaky_relu(scores, 0.01)) ; attn = e / (rowsum(e)+1e-5)
    x_a = attn @ h ; x_b = adj_b @ (x @ W_gcnb) + b_gcnb
    out = sigmoid(attn_a * x_a + attn_b * x_b)

v2 strategy (per core, NL=768 local rows):
  - Phase A is SHARDED: each core computes HX = x_local @ R for only its
    768 rows (R = [W_sa | s_l w | s_r w | Wa1 | Wa2 | Wb2]), then a single
    AllGather of the [6,128,520] bf16 block replicates h + per-node stats
    (s_r, va) to every core. Local stats (s_l, wa2x, wb2x) stay f32.
  - GCN branch reassociated: t_b = adj_b @ x computed TRANSPOSED with
    x j-tiles as PE weights and the adjacency streaming (this avoids
    needing xw = x @ W_gcnb on all nodes), then a small epilogue
    x_b = t_bT.T @ W_gcnb. The gb gate rides the epilogue as appended
    1-column matmuls (gb = t_b @ Wb1) reusing loaded weights.
  - Attention rowsum rides the x_a matmuls as appended 1-column matmuls
    against a ones vector (weights e_t already loaded), killing the
    768-cycle rowsum stream. ga keeps a 768-cycle va^T @ adj_a stream.
  - e computed in transposed layout [j(part), i(free)]; adjacency passed
    pre-transposed per-core (columns = local rows, global j order).
  - float32r phase-A matmuls; bf16 everywhere else.
"""

import numpy as np
from contextlib import ExitStack

import concourse.bass as bass
import concourse.bacc as bacc
import concourse.mybir as mybir
import concourse.tile as tile

F32 = mybir.dt.float32
F32R = mybir.dt.float32r
BF16 = mybir.dt.bfloat16
AF = mybir.ActivationFunctionType
ALU = mybir.AluOpType

N_CORES = 8


def build_program(n, d, nl, ba, bb, dt_a=F32R, dt_bc=BF16, dt_h=BF16,
                  dt_g=None, use_ag=True, rs_append=True, lrelu_on_act=False):
    """Build the SPMD Bass program. Returns nc.

    n: total nodes, d: feature dim, nl: local rows per core.
    ba/bb: python-float gate biases (baked in).
    use_ag: shard phase A + AllGather; else replicate phase A.
    rs_append: rowsum/gb via appended 1-col matmuls (weights reuse).
    """
    if dt_g is None:
        dt_g = mybir.dt.float16
    JT = n // 128   # j tiles (contraction/node axis)
    IT = nl // 128  # local row tiles
    KT = d // 128   # feature k tiles
    NS = 8          # stats cols: 0=s_l 1=s_r 2=zero 3=va 4=wa2x 5=wb2x
    HP = d + NS     # per-j row pitch of the gathered HX block
    MT = IT if use_ag else JT  # phase-A tiles computed locally

    nc = bacc.Bacc("TRN2", target_bir_lowering=False, debug=False,
                   num_devices=N_CORES)

    xt_dram = nc.dram_tensor("xt", [MT, 128, KT * 128], dt_a, kind="ExternalInput")
    xbf_dram = nc.dram_tensor("xbf", [JT, 128, d], dt_g, kind="ExternalInput")
    r_dram = nc.dram_tensor("rmat", [KT, 128, HP], dt_a, kind="ExternalInput")
    adjat_dram = nc.dram_tensor("adjat", [JT, 128, nl], dt_bc, kind="ExternalInput")
    adjbt_dram = nc.dram_tensor("adjbt", [JT, 128, nl], dt_g, kind="ExternalInput")
    wg_dram = nc.dram_tensor("wg", [KT, 128, d + 1], dt_g, kind="ExternalInput")
    bbias_dram = nc.dram_tensor("bbias", [128, d], F32, kind="ExternalInput")
    ident_dram = nc.dram_tensor("ident", [128, 128], F32, kind="ExternalInput")
    out_dram = nc.dram_tensor("out", [nl, d], F32, kind="ExternalOutput")

    def mm(out, lhsT, rhs, start, stop, skip_group_check=False):
        nc.tensor.matmul(out, lhsT, rhs, start=start, stop=stop,
                         skip_group_check=skip_group_check)

    chn = [(0, 512), (512, nl - 512)] if nl > 512 else [(0, nl)]

    with tile.TileContext(nc) as tc, ExitStack() as ctx:
        const = ctx.enter_context(tc.tile_pool(name="const", bufs=1))

        r_sb = const.tile([128, KT, HP], dt_a, tag="r")
        xbf_sb = const.tile([128, JT * d], dt_g, tag="xbf")
        h_sb = const.tile([128, JT, HP], dt_h, tag="h")
        stats_g = const.tile([128, JT * NS], F32, tag="statsg")
        stats_loc = const.tile([128, IT * NS], F32, tag="statsl")
        slb_sb = const.tile([128, nl], F32, tag="slb")
        ga_acc = const.tile([128, nl], F32, tag="ga_acc")
        sl_row = const.tile([1, nl], F32, tag="sl_row")
        tbT_sb = const.tile([128, KT * nl], dt_g, tag="tbT")
        wg_sb = const.tile([128, KT * (d + 1)], dt_g, tag="wg")
        xb_sb = const.tile([128, IT * d], F32, tag="xb")
        gb_sb = const.tile([128, 8], F32, tag="gb")
        ga_row = const.tile([1, nl], F32, tag="ga_row")
        rg_rows = const.tile([2, nl], F32, tag="rg_rows")
        bbias_sb = const.tile([128, d], F32, tag="bbias")
        ident_sb = const.tile([128, 128], F32, tag="ident")
        ones_row = const.tile([1, 128], F32, tag="ones_r")
        one1 = const.tile([1, 1], F32, tag="one1")
        ones_colf = const.tile([128, 1], F32, tag="ones_cf")
        ones_colb = const.tile([128, 1], dt_bc, tag="ones_cb")
        onespad_f = const.tile([128, 2], F32, tag="onespad_f")
        onespad = const.tile([128, 2], dt_bc, tag="onespad")
        neg1 = const.tile([128, 1], F32, tag="neg1")
        ba_sb = const.tile([128, 1], F32, tag="ba")
        bb_sb = const.tile([128, 1], F32, tag="bb")
        gate_sb = const.tile([128, 4 * IT], F32, tag="gate")
        # gate_sb cols: [0:IT]=recip(rowsum), [IT:2IT]=sig_a, [2IT:3IT]=sig_b,
        # [3IT:4IT]=scratch

        # Only phase A's own inputs go first on the sync queue; everything
        # phase B/C needs is issued after phase A's xt DMAs (below) so the
        # first PE matmul isn't stuck behind 6 MB of xbf traffic.
        for k in range(KT):
            nc.sync.dma_start(out=r_sb[:, k, :], in_=r_dram[k])
        nc.sync.dma_start(out=ident_sb[:], in_=ident_dram[:])
        nc.vector.memset(ones_row[:], 1.0)
        nc.vector.memset(one1[:], 1.0)
        nc.vector.memset(ones_colf[:], 1.0)
        nc.vector.tensor_copy(ones_colb[:], ones_colf[:])
        nc.vector.memset(onespad_f[:], 0.0)
        nc.vector.memset(onespad_f[:, 0:1], 1.0)
        nc.vector.tensor_copy(onespad[:], onespad_f[:])
        nc.vector.memset(neg1[:], -1.0)
        nc.vector.memset(ba_sb[:], float(ba))
        nc.vector.memset(bb_sb[:], float(bb))

        dramp = ctx.enter_context(
            tc.tile_pool(name="dram", bufs=1, space="DRAM"))
        if use_ag:
            hx_loc = dramp.tile([IT, 128, HP], dt_h, tag="hx_loc",
                                name="hx_loc")
            hx_full = dramp.tile([JT, 128, HP], dt_h, tag="hx_full",
                                 name="hx_full", addr_space="Shared")

        # ---- Phase A: HX = x @ R for local (or all) rows ----
        with tc.tile_pool(name="xt_pool", bufs=2) as xtp, \
             tc.tile_pool(name="hx_out", bufs=2) as hxp, \
             tc.tile_pool(name="psA", bufs=2, space="PSUM") as psA:
            for m in range(MT):
                xt_t = xtp.tile([128, KT * 128], dt_a, tag="xt")
                nc.sync.dma_start(out=xt_t[:], in_=xt_dram[m])
                ph = psA.tile([128, d], F32, tag="ph")
                ps = psA.tile([128, NS], F32, tag="ps")
                for k in range(KT):
                    lhsT = xt_t[:, k * 128:(k + 1) * 128]
                    st, sp = (k == 0), (k == KT - 1)
                    mm(ph[:], lhsT, r_sb[:, k, 0:d], st, sp)
                    mm(ps[:], lhsT, r_sb[:, k, d:HP], st, sp)
                if m < IT:
                    nc.vector.tensor_copy(stats_loc[:, m * NS:(m + 1) * NS],
                                          ps[:])
                if use_ag:
                    hx_t = hxp.tile([128, HP], dt_h, tag="hx")
                    nc.scalar.copy(hx_t[:, 0:d], ph[:])
                    nc.vector.tensor_copy(hx_t[:, d:HP], ps[:])
                    nc.gpsimd.dma_start(out=hx_loc[m], in_=hx_t[:])
                else:
                    nc.scalar.copy(h_sb[:, m, 0:d], ph[:])
                    nc.vector.tensor_copy(h_sb[:, m, d:HP], ps[:])

        for j in range(JT):
            nc.sync.dma_start(out=xbf_sb[:, j * d:(j + 1) * d],
                              in_=xbf_dram[j])
        for k in range(KT):
            nc.sync.dma_start(out=wg_sb[:, k * (d + 1):(k + 1) * (d + 1)],
                              in_=wg_dram[k])
        nc.sync.dma_start(out=bbias_sb[:], in_=bbias_dram[:])

        if use_ag:
            nc.gpsimd.collective_compute(
                "AllGather",
                mybir.AluOpType.bypass,
                replica_groups=[list(range(N_CORES))],
                ins=[hx_loc.opt()],
                outs=[hx_full.opt()],
            )
            # gpsimd queue: ordered after the collective; does not block the
            # sync-queue adjacency streams.
            for j in range(JT):
                nc.gpsimd.dma_start(out=h_sb[:, j, :], in_=hx_full[j])

        # ---- Phase A2: build SL broadcast [128, nl] from local s_l ----
        with tc.tile_pool(name="psA2", bufs=1, space="PSUM") as psA2:
            pslc = [psA2.tile([1, c[1]], F32, tag=f"psl{ci}",
                              name=f"psl{ci}")
                    for ci, c in enumerate(chn)]
            for t in range(IT):
                ci, off = divmod(t * 128, 512)
                nc.tensor.matmul(pslc[ci][0:1, off:off + 128],
                                 stats_loc[:, t * NS:t * NS + 1],
                                 ident_sb[:], start=True, stop=True)
            for ci, (o, w) in enumerate(chn):
                nc.vector.tensor_copy(sl_row[0:1, o:o + w], pslc[ci][0:1, :])
            for ci, (o, w) in enumerate(chn):
                pbb = psA2.tile([128, w], F32, tag="pbb")
                nc.tensor.matmul(pbb[:], ones_row[:], sl_row[0:1, o:o + w],
                                 start=True, stop=True)
                nc.vector.tensor_copy(slb_sb[:, o:o + w], pbb[:])

        # ---- Phase B: t_bT = (adj_b @ x)^T via x j-tiles as weights ----
        with tc.tile_pool(name="adjB", bufs=4) as adjp, \
             tc.tile_pool(name="psB", bufs=1, space="PSUM") as psB:
            pt_acc = [[psB.tile([128, w], F32, tag=f"pt{dc}_{ci}",
                                name=f"pt{dc}_{ci}")
                       for ci, (o, w) in enumerate(chn)]
                      for dc in range(KT)]
            for j in range(JT):
                at = adjp.tile([128, nl], dt_g, tag="adj")
                nc.scalar.dma_start(out=at[:], in_=adjbt_dram[j])
                st, sp = (j == 0), (j == JT - 1)
                for dc in range(KT):
                    w_ap = xbf_sb[:, j * d + dc * 128:j * d + (dc + 1) * 128]
                    for ci, (o, w) in enumerate(chn):
                        mm(pt_acc[dc][ci][:], w_ap, at[:, o:o + w], st, sp)
            for dc in range(KT):
                for ci, (o, w) in enumerate(chn):
                    nc.scalar.copy(tbT_sb[:, dc * nl + o:dc * nl + o + w],
                                   pt_acc[dc][ci][:])

        # ---- Phase B epilogue: x_b = t_bT.T @ W_gcnb ; gb = t_b @ Wb1 ----
        with tc.tile_pool(name="psE", bufs=1, space="PSUM") as psE:
            pxb = [psE.tile([128, d], F32, tag=f"pxb{i}", name=f"pxb{i}")
                   for i in range(IT)]
            pgb = psE.tile([128, 8], F32, tag="pgb")
            for i in range(IT):
                for k in range(KT):
                    lhsT = tbT_sb[:, k * nl + i * 128:k * nl + (i + 1) * 128]
                    st, sp = (k == 0), (k == KT - 1)
                    mm(pxb[i][:], lhsT, wg_sb[:, k * (d + 1):k * (d + 1) + d],
                       st, sp)
                    mm(pgb[:, i:i + 1], lhsT,
                       wg_sb[:, k * (d + 1) + d:(k + 1) * (d + 1)],
                       st, sp, skip_group_check=True)
            for i in range(IT):
                nc.scalar.copy(xb_sb[:, i * d:(i + 1) * d], pxb[i][:])
            nc.vector.tensor_copy(gb_sb[:], pgb[:])

        # f32 copy of the global stats columns (DVE scalar operands need
        # f32). Placed after phase B so the vector queue isn't head-of-line
        # blocked waiting for the AllGather while A2/B still need it.
        nc.vector.tensor_copy(stats_g[:], h_sb[:, :, d:HP])

        # ---- Phase C: e = adj_a * exp(-lrelu(s)); x_a raw = e^T.T @ h ----
        # rowsum rides the x_a matmuls as appended 1-col matmuls (same
        # weights, own PSUM bank: concurrent accumulation groups must not
        # share a bank with other writers). ga accumulates on the DVE
        # (ga_acc += va * at per j), then 6 one-shot column-reduce matmuls.
        with tc.tile_pool(name="adjC", bufs=4) as adjp2, \
             tc.tile_pool(name="ewC", bufs=4) as ewp, \
             tc.tile_pool(name="psC", bufs=1, space="PSUM") as psC, \
             tc.tile_pool(name="outD", bufs=2) as outp:
            pc = [psC.tile([128, d], F32, tag=f"pc{i}", name=f"pc{i}")
                  for i in range(IT)]
            if rs_append:
                prs = psC.tile([128, 8], F32, tag="prs")
                pgacol = psC.tile([128, 8], F32, tag="pgacol")
                nc.vector.memset(ga_acc[:], 0.0)
            else:
                pga0 = psC.tile([2, 512], F32, tag="pga0")
                pmisc = psC.tile([128, 512], F32, tag="pmisc")
                pga = [pga0, pmisc[0:2, 0:nl - 512]]
                pT2 = pmisc[:, 272:272 + 2 * IT]
            for j in range(JT):
                at = adjp2.tile([128, nl], dt_bc, tag="adj")
                nc.sync.dma_start(out=at[:], in_=adjat_dram[j])
                s_r = stats_g[:, j * NS + 1:j * NS + 2]
                m_t = ewp.tile([128, nl], dt_bc if lrelu_on_act else F32,
                               tag="m")
                if lrelu_on_act:
                    nc.scalar.activation(m_t[:], slb_sb[:], AF.Prelu,
                                         bias=s_r, alpha=0.01)
                else:
                    nc.vector.tensor_scalar_add(m_t[:], slb_sb[:], s_r)
                    nc.vector.scalar_tensor_tensor(m_t[:], m_t[:], 0.01,
                                                   m_t[:], op0=ALU.mult,
                                                   op1=ALU.max)
                nc.scalar.activation(m_t[:], m_t[:], AF.Exp, scale=neg1[:])
                e_t = ewp.tile([128, nl], dt_bc, tag="e")
                nc.vector.tensor_tensor(e_t[:], m_t[:], at[:], op=ALU.mult)
                st, sp = (j == 0), (j == JT - 1)
                for i in range(IT):
                    ew = e_t[:, i * 128:(i + 1) * 128]
                    mm(pc[i][:], ew, h_sb[:, j, 0:d], st, sp)
                    if rs_append:
                        # start=True clears has_written for the WHOLE bank,
                        # so only the very first append may start; the other
                        # columns' first writes overwrite-where-bit-clear.
                        mm(prs[:, i:i + 1], ew, ones_colb[:],
                           st and i == 0, sp, skip_group_check=True)
                if rs_append:
                    va_f = stats_g[:, j * NS + 3:j * NS + 4]
                    nc.vector.scalar_tensor_tensor(ga_acc[:], at[:], va_f,
                                                   ga_acc[:], op0=ALU.mult,
                                                   op1=ALU.add)
                else:
                    zva = h_sb[:, j, d + 2:d + 4]
                    for ci, (o, w) in enumerate(chn):
                        mm(pga[ci][0:2, 0:w], onespad[:], e_t[:, o:o + w],
                           st, False, skip_group_check=True)
                        mm(pga[ci][0:2, 0:w], zva, at[:, o:o + w],
                           False, sp, skip_group_check=True)

            # ---- Phase D: gates + combine (reads PSUM directly) ----
            if rs_append:
                # partition-reduce ga_acc into per-i gate columns
                for i in range(IT):
                    nc.tensor.matmul(pgacol[:, i:i + 1],
                                     ga_acc[:, i * 128:(i + 1) * 128],
                                     ones_colf[:], start=True, stop=True,
                                     skip_group_check=True)
            else:
                # rows 0/1 = rowsum/ga; copy both rows, transpose pairs
                for ci, (o, w) in enumerate(chn):
                    nc.vector.tensor_copy(rg_rows[0:2, o:o + w],
                                          pga[ci][0:2, 0:w])
                for i in range(IT):
                    nc.tensor.matmul(pT2[:, 2 * i:2 * i + 2],
                                     rg_rows[0:2, i * 128:(i + 1) * 128],
                                     ident_sb[0:2, 0:2],
                                     start=True, stop=True,
                                     skip_group_check=True)
            for i in range(IT):
                rs_col = (prs[:, i:i + 1] if rs_append
                          else pT2[:, 2 * i:2 * i + 1])
                ga_col = (pgacol[:, i:i + 1] if rs_append
                          else pT2[:, 2 * i + 1:2 * i + 2])
                nc.vector.tensor_scalar_add(
                    gate_sb[:, 3 * IT + i:3 * IT + i + 1],
                    rs_col, 1e-5)
                nc.vector.reciprocal(gate_sb[:, i:i + 1],
                                     gate_sb[:, 3 * IT + i:3 * IT + i + 1])
                # sig_a = sigmoid(ga + wa2x + ba)
                nc.vector.tensor_tensor(gate_sb[:, 3 * IT + i:3 * IT + i + 1],
                                        ga_col,
                                        stats_loc[:, i * NS + 4:i * NS + 5],
                                        op=ALU.add)
                nc.scalar.activation(gate_sb[:, IT + i:IT + i + 1],
                                     gate_sb[:, 3 * IT + i:3 * IT + i + 1],
                                     AF.Sigmoid, bias=ba_sb[:])
                # sig_b = sigmoid(gb + wb2x + bb)
                nc.vector.tensor_tensor(gate_sb[:, 3 * IT + i:3 * IT + i + 1],
                                        gb_sb[:, i:i + 1],
                                        stats_loc[:, i * NS + 5:i * NS + 6],
                                        op=ALU.add)
                nc.scalar.activation(gate_sb[:, 2 * IT + i:2 * IT + i + 1],
                                     gate_sb[:, 3 * IT + i:3 * IT + i + 1],
                                     AF.Sigmoid, bias=bb_sb[:])
            for i in range(IT):
                u_t = outp.tile([128, d], F32, tag="u")
                # u = sig_a * (x_a_raw * recip)
                nc.vector.tensor_scalar(u_t[:], pc[i][:],
                                        gate_sb[:, i:i + 1],
                                        gate_sb[:, IT + i:IT + i + 1],
                                        op0=ALU.mult, op1=ALU.mult)
                t_t = outp.tile([128, d], F32, tag="t")
                nc.vector.tensor_tensor(t_t[:], xb_sb[:, i * d:(i + 1) * d],
                                        bbias_sb[:], op=ALU.add)
                # y = sigmoid(t * sig_b + u)
                nc.vector.scalar_tensor_tensor(
                    t_t[:], t_t[:], gate_sb[:, 2 * IT + i:2 * IT + i + 1],
                    u_t[:], op0=ALU.mult, op1=ALU.add)
                y_t = outp.tile([128, d], F32, tag="y")
                nc.scalar.activation(y_t[:], t_t[:], AF.Sigmoid)
                nc.sync.dma_start(out=out_dram[i * 128:(i + 1) * 128, :],
                                  in_=y_t[:])

    nc.compile()
    return nc


def make_r_matrix(W_sa, a_sa, Wa, Wb, d):
    cols = np.zeros((d, 8), dtype=np.float32)
    cols[:, 0] = W_sa @ a_sa[0, :d]     # s_l weights
    cols[:, 1] = W_sa @ a_sa[0, d:]     # s_r weights
    # col 2 stays zero (zero-pad for the [0|va] gate weight pair)
    cols[:, 3] = Wa[0, :d]              # va (ga = adj_a @ (x @ Wa1))
    cols[:, 4] = Wa[0, d:]              # wa2x
    cols[:, 5] = Wb[0, d:]              # wb2x
    return np.ascontiguousarray(
        np.concatenate([W_sa, cols], axis=1)).astype(np.float32)


def make_shared_inputs(x, R, W_gcnb, Wb, b_gcnb, n, d, np_a=np.float32,
                       np_g=np.float16):
    JT, KT = n // 128, d // 128
    wg = np.concatenate([W_gcnb, Wb[0, :d][:, None]], axis=1)  # [d, d+1]
    return {
        "xbf": np.ascontiguousarray(x.reshape(JT, 128, d)).astype(np_g),
        "rmat": R.reshape(KT, 128, d + 8).astype(np_a),
        "wg": np.ascontiguousarray(wg.reshape(KT, 128, d + 1)).astype(np_g),
        "bbias": np.ascontiguousarray(
            np.broadcast_to(b_gcnb, (128, d))).astype(np.float32),
        "ident": np.eye(128, dtype=np.float32),
    }


def make_core_inputs(x, adj_a, adj_b, n, d, nl, core, use_ag=True,
                     np_a=np.float32, np_bc=None, np_g=np.float16):
    if np_bc is None:
        import ml_dtypes
        np_bc = ml_dtypes.bfloat16
    JT, KT = n // 128, d // 128
    rows = np.arange(core * nl, (core + 1) * nl)
    if use_ag:
        xl = x[rows]
        MT = nl // 128
    else:
        # local rows first so stats tiles 0..IT-1 are local
        perm = np.concatenate([rows, np.arange(0, core * nl),
                               np.arange((core + 1) * nl, n)])
        xl = x[perm]
        MT = JT
    # [m, kk, k*128+mm] = x[rows[m*128+mm], k*128+kk]
    xt = np.ascontiguousarray(
        xl.reshape(MT, 128, KT, 128).transpose(0, 3, 2, 1)
        .reshape(MT, 128, KT * 128))
    adjat = np.ascontiguousarray(adj_a[rows].T).reshape(JT, 128, nl)
    adjbt = np.ascontiguousarray(adj_b[rows].T).reshape(JT, 128, nl)
    return {
        "xt": xt.astype(np_a),
        "adjat": adjat.astype(np_bc),
        "adjbt": adjbt.astype(np_g),
    }


_CACHE = {}


def _install_ntff_hook():
    """Dev-only: register the axon NTFF profile hook so trace=True works."""
    import sys
    import types
    try:
        from antenv import axon_hooks  # noqa: F401
        return
    except ImportError:
        pass
    import antenv
    mod = types.ModuleType("antenv.axon_hooks")
    _h = [None]
    mod.get_axon_ntff_profile_hook = lambda: _h[0]
    mod.set_axon_ntff_profile_hook = lambda hook: _h.__setitem__(0, hook)
    sys.modules["antenv.axon_hooks"] = mod
    antenv.axon_hooks = mod
    from trn_agent_boot.trn_boot import _ntff_profile_via_ctypes
    mod.set_axon_ntff_profile_hook(
        _ntff_profile_via_ctypes("/opt/axon/libaxon_pjrt.so"))


USE_AG = True
RS_APPEND = True
LRELU_ON_ACT = True


def kernel(x, adj_a, adj_b, W_sa, a_sa, W_gcnb, b_gcnb, Wa, ba, Wb, bb,
           _trace=False, _trace_kwargs=None):
    from concourse.bass_utils import run_bass_kernel_spmd
    if _trace:
        _install_ntff_hook()

    n, d = x.shape
    nl = n // N_CORES
    R = make_r_matrix(W_sa, a_sa, Wa, Wb, d)

    key = (n, d, nl, float(ba[0]), float(bb[0]), USE_AG, RS_APPEND,
           LRELU_ON_ACT)
    if key not in _CACHE:
        _CACHE[key] = build_program(n, d, nl, float(ba[0]), float(bb[0]),
                                    use_ag=USE_AG, rs_append=RS_APPEND,
                                    lrelu_on_act=LRELU_ON_ACT)
    nc = _CACHE[key]

    shared = make_shared_inputs(x, R, W_gcnb, Wb, b_gcnb, n, d)
    in_maps = []
    for c in range(N_CORES):
        m = dict(shared)
        m.update(make_core_inputs(x, adj_a, adj_b, n, d, nl, c,
                                  use_ag=USE_AG))
        in_maps.append(m)
    res = run_bass_kernel_spmd(nc, in_maps, list(range(N_CORES)),
                               trace=_trace, **(_trace_kwargs or {}))
    out = np.empty((n, d), dtype=np.float32)
    for c in range(N_CORES):
        out[c * nl:(c + 1) * nl] = res.results[c]["out"]
    if _trace:
        kernel._last_results = res
    return out


# revision 31
# speedup vs baseline: 1.4748x; 1.0539x over previous
"""HGCN layer kernel for Trainium2, 8 NeuronCores, row-sharded SPMD.

Reference computation (N=6144, D=512):
    type_sum_a = adj_a @ x ; type_sum_b = adj_b @ x
    attn_a = sigmoid(cat[ts_a, x] @ Wa.T + ba) ; attn_b likewise
    h = x @ W_sa ; s_l = h @ a_sa[:512] ; s_r = h @ a_sa[512:]
    scores[i,j] = s_l[i] + s_r[j]
    e = adj_a * exp(-leaky_relu(scores, 0.01)) ; attn = e / (rowsum(e)+1e-5)
    x_a = attn @ h ; x_b = adj_b @ (x @ W_gcnb) + b_gcnb
    out = sigmoid(attn_a * x_a + attn_b * x_b)

v2.4 strategy (per core, NL=768 local rows):
  - Phase A SHARDED (bf16): each core computes HX = x_local @ R for its
    768 rows (R = [W_sa | s_l w | s_r w | 0 | Wa1 | Wa2 | Wb2]), then one
    AllGather of the [128, 6*520] bf16 rank block replicates h + stats.
    Rank-block (partition-major) layout -> 8 batched gather-in DMAs.
  - GCN branch (fp16): t_bT = (adj_b @ x)^T with x j-tiles as PE weights
    and adjacency streaming; epilogue x_b = t_bT.T @ W_gcnb runs AFTER
    phase C (PSUM bank budget); b_gcnb folded in as a rank-1 matmul; the
    gb gate rides as appended 1-col matmuls on loaded weights.
  - Attention: e in transposed layout [j(part), i(free)]; Prelu(+s_r
    bias) and Exp on the Scalar engine (one act-table set), mask-mult on
    DVE; rowsum rides the x_a matmuls as appended 1-col matmuls into an
    exclusive PSUM bank (start=True clears has_written for the WHOLE
    bank, so only the first append starts); ga accumulates on the DVE
    (ga_acc += va*adj_a per j) + 6 one-shot column-reduce matmuls.
  - DMA queues: sync = xt | interleaved xbf/adjbt | adjat | out;
    scalar = (free for phase C runahead); gpsimd = hx out, AllGather,
    8 batched h-block DMAs. Phase C's scalar chain runs ahead during
    phase B as soon as the gather lands.
"""

import numpy as np
from contextlib import ExitStack

import concourse.bass as bass
import concourse.bacc as bacc
import concourse.mybir as mybir
import concourse.tile as tile

F32 = mybir.dt.float32
F32R = mybir.dt.float32r
BF16 = mybir.dt.bfloat16
FP16 = mybir.dt.float16
AF = mybir.ActivationFunctionType
ALU = mybir.AluOpType

N_CORES = 8


def build_program(n, d, nl, ba, bb, dt_a=BF16, dt_bc=BF16, dt_h=BF16,
                  dt_g=FP16, use_ag=True, rs_append=True,
                  lrelu_on_act=True):
    """Build the SPMD Bass program. Returns nc."""
    JT = n // 128   # j tiles (contraction/node axis)
    IT = nl // 128  # local row tiles
    KT = d // 128   # feature k tiles
    NS = 8          # stats cols: 0=s_l 1=s_r 2=zero 3=va 4=wa2x 5=wb2x
    HP = d + NS     # per-j pitch of the gathered HX block
    MT = IT if use_ag else JT

    nc = bacc.Bacc("TRN2", target_bir_lowering=False, debug=False,
                   num_devices=N_CORES)

    xt_dram = nc.dram_tensor("xt", [MT, 128, KT * 128], dt_a, kind="ExternalInput")
    xbf_dram = nc.dram_tensor("xbf", [JT, 128, d], dt_g, kind="ExternalInput")
    r_dram = nc.dram_tensor("rmat", [128, KT * HP], dt_a, kind="ExternalInput")
    adjat_dram = nc.dram_tensor("adjat", [JT, 128, nl], dt_bc, kind="ExternalInput")
    adjbt_dram = nc.dram_tensor("adjbt", [JT, 128, nl], dt_g, kind="ExternalInput")
    wg_dram = nc.dram_tensor("wg", [KT, 128, d + 1], dt_g, kind="ExternalInput")
    brow_dram = nc.dram_tensor("brow", [1, d], dt_g, kind="ExternalInput")
    ident_dram = nc.dram_tensor("ident", [128, 128], F32, kind="ExternalInput")
    out_dram = nc.dram_tensor("out", [nl, d], F32, kind="ExternalOutput")

    def mm(out, lhsT, rhs, start, stop, skip_group_check=False):
        nc.tensor.matmul(out, lhsT, rhs, start=start, stop=stop,
                         skip_group_check=skip_group_check)

    chn = [(0, 512), (512, nl - 512)] if nl > 512 else [(0, nl)]

    with tile.TileContext(nc) as tc, ExitStack() as ctx:
        const = ctx.enter_context(tc.tile_pool(name="const", bufs=1))

        r_sb = const.tile([128, KT, HP], dt_a, tag="r")
        xbf_sb = const.tile([128, JT * d], dt_g, tag="xbf")
        h_sb = const.tile([128, JT, HP], dt_h, tag="h")
        stats_g = const.tile([128, JT * NS], F32, tag="statsg")
        stats_loc = const.tile([128, IT * NS], F32, tag="statsl")
        slb_sb = const.tile([128, nl], F32, tag="slb")
        ga_acc = const.tile([128, nl], F32, tag="ga_acc")
        sl_row = const.tile([1, nl], F32, tag="sl_row")
        tbT_sb = const.tile([128, KT * nl], dt_g, tag="tbT")
        wg_sb = const.tile([128, KT * (d + 1)], dt_g, tag="wg")
        brow_sb = const.tile([1, d], dt_g, tag="brow")
        u_sb = const.tile([128, IT * d], F32, tag="u")
        ident_sb = const.tile([128, 128], F32, tag="ident")
        ones_row = const.tile([1, 128], F32, tag="ones_r")
        ones_16 = const.tile([1, 128], dt_g, tag="ones16")
        ones_colf = const.tile([128, 1], F32, tag="ones_cf")
        ones_colb = const.tile([128, 1], dt_bc, tag="ones_cb")
        neg1 = const.tile([128, 1], F32, tag="neg1")
        ba_sb = const.tile([128, 1], F32, tag="ba")
        bb_sb = const.tile([128, 1], F32, tag="bb")
        gate_sb = const.tile([128, 4 * IT], F32, tag="gate")
        # gate cols: [0:IT]=recip(rowsum) [IT:2IT]=sig_a [2IT:3IT]=sig_b
        # [3IT:4IT]=scratch

        nc.sync.dma_start(out=r_sb.opt(), in_=r_dram[:])
        nc.sync.dma_start(out=ident_sb[:], in_=ident_dram[:])
        nc.vector.memset(ones_row[:], 1.0)
        nc.vector.tensor_copy(ones_16[:], ones_row[:])
        nc.vector.memset(ones_colf[:], 1.0)
        nc.vector.tensor_copy(ones_colb[:], ones_colf[:])
        nc.vector.memset(neg1[:], -1.0)
        nc.vector.memset(ba_sb[:], float(ba))
        nc.vector.memset(bb_sb[:], float(bb))

        dramp = ctx.enter_context(
            tc.tile_pool(name="dram", bufs=1, space="DRAM"))
        if use_ag:
            # partition-major rank block: AG output = [rank][128][IT*HP]
            hx_loc = dramp.tile([128, IT * HP], dt_h, tag="hx_loc",
                                name="hx_loc")
            hx_full = dramp.tile([N_CORES, 128, IT * HP], dt_h,
                                 tag="hx_full", name="hx_full",
                                 addr_space="Shared")

        # ---- Phase A: HX = x @ R for local rows (bf16) ----
        with tc.tile_pool(name="xt_pool", bufs=2) as xtp, \
             tc.tile_pool(name="hx_out", bufs=2) as hxp, \
             tc.tile_pool(name="psA", bufs=2, space="PSUM") as psA:
            for m in range(MT):
                xt_t = xtp.tile([128, KT * 128], dt_a, tag="xt")
                nc.sync.dma_start(out=xt_t[:], in_=xt_dram[m])
                ph = psA.tile([128, d], F32, tag="ph")
                ps = psA.tile([128, NS], F32, tag="ps")
                for k in range(KT):
                    lhsT = xt_t[:, k * 128:(k + 1) * 128]
                    st, sp = (k == 0), (k == KT - 1)
                    mm(ph[:], lhsT, r_sb[:, k, 0:d], st, sp)
                    mm(ps[:], lhsT, r_sb[:, k, d:HP], st, sp)
                if m < IT:
                    nc.vector.tensor_copy(stats_loc[:, m * NS:(m + 1) * NS],
                                          ps[:])
                if use_ag:
                    hx_t = hxp.tile([128, HP], dt_h, tag="hx")
                    nc.scalar.copy(hx_t[:, 0:d], ph[:])
                    nc.vector.tensor_copy(hx_t[:, d:HP], ps[:])
                    nc.gpsimd.dma_start(
                        out=hx_loc[:, m * HP:(m + 1) * HP], in_=hx_t[:])
                else:
                    nc.scalar.copy(h_sb[:, m, 0:d], ph[:])
                    nc.vector.tensor_copy(h_sb[:, m, d:HP], ps[:])

        if use_ag:
            nc.gpsimd.collective_compute(
                "AllGather",
                mybir.AluOpType.bypass,
                replica_groups=[list(range(N_CORES))],
                ins=[hx_loc.opt()],
                outs=[hx_full.opt()],
            )
            # 8 batched rank-block DMAs on the gpsimd queue (ordered after
            # the collective; sync queue stays free for adjacency).
            for r in range(N_CORES):
                nc.gpsimd.dma_start(
                    out=h_sb[:, r * IT:(r + 1) * IT, :], in_=hx_full[r])

        # ---- Phase A2: build SL broadcast [128, nl] from local s_l ----
        with tc.tile_pool(name="psA2", bufs=1, space="PSUM") as psA2:
            pslc = [psA2.tile([1, c[1]], F32, tag=f"psl{ci}",
                              name=f"psl{ci}")
                    for ci, c in enumerate(chn)]
            for t in range(IT):
                ci, off = divmod(t * 128, 512)
                nc.tensor.matmul(pslc[ci][0:1, off:off + 128],
                                 stats_loc[:, t * NS:t * NS + 1],
                                 ident_sb[:], start=True, stop=True)
            for ci, (o, w) in enumerate(chn):
                nc.vector.tensor_copy(sl_row[0:1, o:o + w], pslc[ci][0:1, :])
            for ci, (o, w) in enumerate(chn):
                pbb = psA2.tile([128, w], F32, tag="pbb")
                nc.tensor.matmul(pbb[:], ones_row[:], sl_row[0:1, o:o + w],
                                 start=True, stop=True)
                nc.vector.tensor_copy(slb_sb[:, o:o + w], pbb[:])

        # per-rank-block f32 stats casts (progressive; unblocks phase C's
        # scalar runahead as soon as each gathered block lands)
        for r in range(N_CORES):
            nc.vector.tensor_copy(
                stats_g[:, r * IT * NS:(r + 1) * IT * NS],
                h_sb[:, r * IT:(r + 1) * IT, d:HP])
        nc.vector.memset(ga_acc[:], 0.0)

        # ---- Phase B: t_bT = (adj_b @ x)^T, x j-tiles as weights ----
        # xbf/adjbt interleaved per-j on the sync queue: phase B starts as
        # soon as the first pair lands instead of waiting for all of xbf.
        with tc.tile_pool(name="adjB", bufs=4) as adjp, \
             tc.tile_pool(name="psB", bufs=1, space="PSUM") as psB:
            pt_acc = [[psB.tile([128, w], F32, tag=f"pt{dc}_{ci}",
                                name=f"pt{dc}_{ci}")
                       for ci, (o, w) in enumerate(chn)]
                      for dc in range(KT)]
            for j in range(JT):
                nc.sync.dma_start(out=xbf_sb[:, j * d:(j + 1) * d],
                                  in_=xbf_dram[j])
                at = adjp.tile([128, nl], dt_g, tag="adj")
                nc.sync.dma_start(out=at[:], in_=adjbt_dram[j])
                st, sp = (j == 0), (j == JT - 1)
                for dc in range(KT):
                    w_ap = xbf_sb[:, j * d + dc * 128:j * d + (dc + 1) * 128]
                    for ci, (o, w) in enumerate(chn):
                        mm(pt_acc[dc][ci][:], w_ap, at[:, o:o + w], st, sp)
            for dc in range(KT):
                for ci, (o, w) in enumerate(chn):
                    nc.vector.tensor_copy(
                        tbT_sb[:, dc * nl + o:dc * nl + o + w],
                        pt_acc[dc][ci][:])

        for k in range(KT):
            nc.sync.dma_start(out=wg_sb[:, k * (d + 1):(k + 1) * (d + 1)],
                              in_=wg_dram[k])
        nc.sync.dma_start(out=brow_sb[:], in_=brow_dram[:])

        # ---- Phase C + gates-a + u ----
        with tc.tile_pool(name="adjC", bufs=4) as adjp2, \
             tc.tile_pool(name="mC", bufs=10) as mp, \
             tc.tile_pool(name="eC", bufs=4) as ep, \
             tc.tile_pool(name="psC", bufs=1, space="PSUM") as psC:
            pc = [psC.tile([128, d], F32, tag=f"pc{i}", name=f"pc{i}")
                  for i in range(IT)]
            prs = psC.tile([128, 8], F32, tag="prs")
            pgacol = psC.tile([128, 8], F32, tag="pgacol")
            for j in range(JT):
                at = adjp2.tile([128, nl], dt_bc, tag="adj")
                nc.sync.dma_start(out=at[:], in_=adjat_dram[j])
                s_r = stats_g[:, j * NS + 1:j * NS + 2]
                m_t = mp.tile([128, nl], dt_bc if lrelu_on_act else F32,
                              tag="m")
                if lrelu_on_act:
                    nc.scalar.activation(m_t[:], slb_sb[:], AF.Prelu,
                                         bias=s_r, alpha=0.01)
                else:
                    nc.vector.tensor_scalar_add(m_t[:], slb_sb[:], s_r)
                    nc.vector.scalar_tensor_tensor(m_t[:], m_t[:], 0.01,
                                                   m_t[:], op0=ALU.mult,
                                                   op1=ALU.max)
                nc.scalar.activation(m_t[:], m_t[:], AF.Exp, scale=neg1[:])
                e_t = ep.tile([128, nl], dt_bc, tag="e")
                nc.vector.tensor_tensor(e_t[:], m_t[:], at[:], op=ALU.mult)
                va_f = stats_g[:, j * NS + 3:j * NS + 4]
                nc.vector.scalar_tensor_tensor(ga_acc[:], at[:], va_f,
                                               ga_acc[:], op0=ALU.mult,
                                               op1=ALU.add)
                st, sp = (j == 0), (j == JT - 1)
                for i in range(IT):
                    ew = e_t[:, i * 128:(i + 1) * 128]
                    mm(pc[i][:], ew, h_sb[:, j, 0:d], st, sp)
                    # start=True clears has_written for the WHOLE bank:
                    # only the very first append may start.
                    mm(prs[:, i:i + 1], ew, ones_colb[:],
                       st and i == 0, sp, skip_group_check=True)

            # ga partition-reduction into per-i gate columns
            for i in range(IT):
                nc.tensor.matmul(pgacol[:, i:i + 1],
                                 ga_acc[:, i * 128:(i + 1) * 128],
                                 ones_colf[:], start=True, stop=True,
                                 skip_group_check=True)
            # gates-a + u = sig_a * recip * x_a_raw (PSUM freed after)
            for i in range(IT):
                nc.vector.tensor_scalar_add(
                    gate_sb[:, 3 * IT + i:3 * IT + i + 1],
                    prs[:, i:i + 1], 1e-5)
                nc.vector.reciprocal(gate_sb[:, i:i + 1],
                                     gate_sb[:, 3 * IT + i:3 * IT + i + 1])
                nc.vector.tensor_tensor(gate_sb[:, 3 * IT + i:3 * IT + i + 1],
                                        pgacol[:, i:i + 1],
                                        stats_loc[:, i * NS + 4:i * NS + 5],
                                        op=ALU.add)
                nc.scalar.activation(gate_sb[:, IT + i:IT + i + 1],
                                     gate_sb[:, 3 * IT + i:3 * IT + i + 1],
                                     AF.Sigmoid, bias=ba_sb[:])
            for i in range(IT):
                nc.vector.tensor_scalar(u_sb[:, i * d:(i + 1) * d], pc[i][:],
                                        gate_sb[:, i:i + 1],
                                        gate_sb[:, IT + i:IT + i + 1],
                                        op0=ALU.mult, op1=ALU.mult)

        # ---- Epilogue: x_b = t_bT.T @ W_gcnb + b ; gb = t_b @ Wb1 ----
        with tc.tile_pool(name="psE", bufs=1, space="PSUM") as psE, \
             tc.tile_pool(name="outD", bufs=2) as outp:
            pxb = [psE.tile([128, d], F32, tag=f"pxb{i}", name=f"pxb{i}")
                   for i in range(IT)]
            pgb = psE.tile([128, 8], F32, tag="pgb")
            for i in range(IT):
                for k in range(KT):
                    lhsT = tbT_sb[:, k * nl + i * 128:k * nl + (i + 1) * 128]
                    mm(pxb[i][:], lhsT, wg_sb[:, k * (d + 1):k * (d + 1) + d],
                       k == 0, False)
                    mm(pgb[:, i:i + 1], lhsT,
                       wg_sb[:, k * (d + 1) + d:(k + 1) * (d + 1)],
                       k == 0, k == KT - 1, skip_group_check=True)
                # fold b_gcnb in as a rank-1 update (ones x brow)
                mm(pxb[i][:], ones_16[:], brow_sb[:], False, True)

            # ---- Phase D2: sig_b gate + combine + output ----
            for i in range(IT):
                nc.vector.tensor_tensor(gate_sb[:, 3 * IT + i:3 * IT + i + 1],
                                        pgb[:, i:i + 1],
                                        stats_loc[:, i * NS + 5:i * NS + 6],
                                        op=ALU.add)
                nc.scalar.activation(gate_sb[:, 2 * IT + i:2 * IT + i + 1],
                                     gate_sb[:, 3 * IT + i:3 * IT + i + 1],
                                     AF.Sigmoid, bias=bb_sb[:])
            for i in range(IT):
                t_t = outp.tile([128, d], F32, tag="t")
                # y = sigmoid(x_b * sig_b + u)
                nc.vector.scalar_tensor_tensor(
                    t_t[:], pxb[i][:], gate_sb[:, 2 * IT + i:2 * IT + i + 1],
                    u_sb[:, i * d:(i + 1) * d], op0=ALU.mult, op1=ALU.add)
                y_t = outp.tile([128, d], F32, tag="y")
                nc.scalar.activation(y_t[:], t_t[:], AF.Sigmoid)
                nc.sync.dma_start(out=out_dram[i * 128:(i + 1) * 128, :],
                                  in_=y_t[:])

    nc.compile()
    return nc


def make_r_matrix(W_sa, a_sa, Wa, Wb, d):
    cols = np.zeros((d, 8), dtype=np.float32)
    cols[:, 0] = W_sa @ a_sa[0, :d]     # s_l weights
    cols[:, 1] = W_sa @ a_sa[0, d:]     # s_r weights
    # col 2 stays zero
    cols[:, 3] = Wa[0, :d]              # va (ga = adj_a @ (x @ Wa1))
    cols[:, 4] = Wa[0, d:]              # wa2x
    cols[:, 5] = Wb[0, d:]              # wb2x
    return np.ascontiguousarray(
        np.concatenate([W_sa, cols], axis=1)).astype(np.float32)


def make_shared_inputs(x, R, W_gcnb, Wb, b_gcnb, n, d, np_a=None,
                       np_g=np.float16):
    import ml_dtypes
    if np_a is None:
        np_a = ml_dtypes.bfloat16
    JT, KT = n // 128, d // 128
    HP = d + 8
    wg = np.concatenate([W_gcnb, Wb[0, :d][:, None]], axis=1)  # [d, d+1]
    rmat = np.ascontiguousarray(
        R.reshape(KT, 128, HP).transpose(1, 0, 2).reshape(128, KT * HP))
    return {
        "xbf": np.ascontiguousarray(x.reshape(JT, 128, d)).astype(np_g),
        "rmat": rmat.astype(np_a),
        "wg": np.ascontiguousarray(wg.reshape(KT, 128, d + 1)).astype(np_g),
        "brow": b_gcnb[None, :].astype(np_g),
        "ident": np.eye(128, dtype=np.float32),
    }


def make_core_inputs(x, adj_a, adj_b, n, d, nl, core, use_ag=True,
                     np_a=None, np_bc=None, np_g=np.float16):
    import ml_dtypes
    if np_a is None:
        np_a = ml_dtypes.bfloat16
    if np_bc is None:
        np_bc = ml_dtypes.bfloat16
    JT, KT = n // 128, d // 128
    rows = np.arange(core * nl, (core + 1) * nl)
    if use_ag:
        xl = x[rows]
        MT = nl // 128
    else:
        perm = np.concatenate([rows, np.arange(0, core * nl),
                               np.arange((core + 1) * nl, n)])
        xl = x[perm]
        MT = JT
    # [m, kk, k*128+mm] = x[rows[m*128+mm], k*128+kk]
    xt = np.ascontiguousarray(
        xl.reshape(MT, 128, KT, 128).transpose(0, 3, 2, 1)
        .reshape(MT, 128, KT * 128))
    adjat = np.ascontiguousarray(adj_a[rows].T).reshape(JT, 128, nl)
    adjbt = np.ascontiguousarray(adj_b[rows].T).reshape(JT, 128, nl)
    return {
        "xt": xt.astype(np_a),
        "adjat": adjat.astype(np_bc),
        "adjbt": adjbt.astype(np_g),
    }


_CACHE = {}


def _install_ntff_hook():
    """Dev-only: register the axon NTFF profile hook so trace=True works."""
    import sys
    import types
    try:
        from antenv import axon_hooks  # noqa: F401
        return
    except ImportError:
        pass
    import antenv
    mod = types.ModuleType("antenv.axon_hooks")
    _h = [None]
    mod.get_axon_ntff_profile_hook = lambda: _h[0]
    mod.set_axon_ntff_profile_hook = lambda hook: _h.__setitem__(0, hook)
    sys.modules["antenv.axon_hooks"] = mod
    antenv.axon_hooks = mod
    from trn_agent_boot.trn_boot import _ntff_profile_via_ctypes
    mod.set_axon_ntff_profile_hook(
        _ntff_profile_via_ctypes("/opt/axon/libaxon_pjrt.so"))


USE_AG = True
RS_APPEND = True
LRELU_ON_ACT = True


def kernel(x, adj_a, adj_b, W_sa, a_sa, W_gcnb, b_gcnb, Wa, ba, Wb, bb,
           _trace=False, _trace_kwargs=None):
    from concourse.bass_utils import run_bass_kernel_spmd
    if _trace:
        _install_ntff_hook()

    n, d = x.shape
    nl = n // N_CORES
    R = make_r_matrix(W_sa, a_sa, Wa, Wb, d)

    key = (n, d, nl, float(ba[0]), float(bb[0]), USE_AG, RS_APPEND,
           LRELU_ON_ACT, "v24")
    if key not in _CACHE:
        _CACHE[key] = build_program(n, d, nl, float(ba[0]), float(bb[0]),
                                    use_ag=USE_AG, rs_append=RS_APPEND,
                                    lrelu_on_act=LRELU_ON_ACT)
    nc = _CACHE[key]

    shared = make_shared_inputs(x, R, W_gcnb, Wb, b_gcnb, n, d)
    in_maps = []
    for c in range(N_CORES):
        m = dict(shared)
        m.update(make_core_inputs(x, adj_a, adj_b, n, d, nl, c,
                                  use_ag=USE_AG))
        in_maps.append(m)
    res = run_bass_kernel_spmd(nc, in_maps, list(range(N_CORES)),
                               trace=_trace, **(_trace_kwargs or {}))
    out = np.empty((n, d), dtype=np.float32)
    for c in range(N_CORES):
        out[c * nl:(c + 1) * nl] = res.results[c]["out"]
    if _trace:
        kernel._last_results = res
    return out
